# revision 1
# baseline (speedup 1.0000x reference)
"""Trainium2 Bass kernel for nn_Net_34763465294339.

Four single-channel VALID convs (K=25/49/97/193, 16 output channels each) on
x[16,1,256,256], each squared + spatially averaged / scale -> stack -> fold
16 channels into 8 by adding halves. Output [16,8,4] f32.

Sharding: data-parallel over batch, 2 images per core, weights replicated.

Conv-as-matmul (output-stationary):
  PSUM tile per 8-output-row block: partitions m=(s,o)=8x16=128, free
  n=(j,b)=2*S (both images column-interleaved). Contraction k=(t,dj) over T kernel
  rows (T*K<=128; K=193 splits dj into 2 chunks). Accumulate over base-row
  sweep q=0..Q-1 (r0=i0+q*T) in PSUM.

  All per-q weight matrices are AP-offset slices of one padded matrix per
  chunk: M[(t,dj),(u,o)] = w[o, t+qmaxT-u, dj] (zero outside [0,K)), with
  lhsT_q = M[:, u0:u0+8, :] at u0=qmaxT-q*T.

  rhs tiles are shifted-row im2col tiles DMA'd straight from DRAM with
  overlapping-read APs, rotating through a small pool (re-fetched per group).

  Post: per block, DVE tensor_tensor_reduce (square, scaled, free-dim sum)
  into a stage column; a tiny fp32 fold-matmul (ones matrix) folds the
  (s,o)->o%8 partitions; per-(conv,image) column reduce; one DMA out.
"""
import os

import numpy as np
import ml_dtypes

import concourse.bass as bass
import concourse.bacc as bacc
import concourse.mybir as mybir
from concourse.tile import TileContext
from concourse.bass_utils import run_bass_kernel_spmd

BF16 = mybir.dt.bfloat16
F32 = mybir.dt.float32

IMG = 256
NCORES = 8
BLOCK_I = 8  # output rows per psum block
GROUP = 8    # psum blocks in flight (8 PSUM banks)

# (K, T, scale)
CONVS = [(25, 4, 1.0), (49, 2, 2.0), (97, 1, 4.0), (193, 1, 8.0)]
# rhs rotating-pool bufs per conv tag (>= max tiles in flight + prefetch)
RHS_BUFS = {25: 24, 49: 40, 97: 72, 193: 72}


def _conv_cfg(K, T):
    S = IMG - K + 1
    Q = (K + 7) // T
    U = (Q - 1) * T + 8
    chunks = [(0, K)] if T * K <= 128 else [(0, 128), (128, K)]
    return S, Q, U, chunks


def _build_M(w, K, T, scale):
    """w: [16,K,K] fp32, pre-scaled by sqrt(1/(S^2*scale)) so the squared
    conv outputs sum directly to the scaled mean. Returns fp32 [T*Kc, U*16]
    per dj-chunk."""
    S, Q, U, chunks = _conv_cfg(K, T)
    w = w * np.sqrt(1.0 / (float(S) * S * scale), dtype=np.float32)
    qmaxT = (Q - 1) * T
    out = []
    for (lo, hi) in chunks:
        Kc = hi - lo
        M = np.zeros((T * Kc, U, 16), dtype=np.float32)
        for t in range(T):
            for u in range(U):
                di = t + qmaxT - u
                if 0 <= di < K:
                    M[t * Kc:(t + 1) * Kc, u, :] = w[:, di, lo:hi].T
        out.append(np.ascontiguousarray(M.reshape(T * Kc, U * 16)))
    return out


def _build_fold():
    F = np.zeros((128, 8), dtype=np.float32)
    for p in range(128):
        F[p, (p % 16) % 8] = 1.0
    return F


def _col_layout(convs):
    col_base = {}
    c = 0
    for (K, T, scale) in convs:
        nb = (IMG - K + 1) // BLOCK_I
        for b in range(2):
            col_base[(K, b)] = c
            c += nb
    return col_base, c


def _build_nc(convs):
    nc = bacc.Bacc("TRN2", target_bir_lowering=False)
    x = nc.dram_tensor("x", [IMG, IMG, 2], BF16, kind="ExternalInput")
    m_handles = {}
    for (K, T, scale) in convs:
        S, Q, U, chunks = _conv_cfg(K, T)
        for idx, (lo, hi) in enumerate(chunks):
            Kc = hi - lo
            m_handles[(K, idx)] = nc.dram_tensor(
                f"m{K}_{idx}", [T * Kc, U * 16], BF16, kind="ExternalInput")
    fold = nc.dram_tensor("fold", [128, 8], F32, kind="ExternalInput")
    out = nc.dram_tensor("out", [2, 8, 4], F32, kind="ExternalOutput")

    col_base, TOT = _col_layout(convs)

    with TileContext(nc) as tc:
        with tc.tile_pool(name="consts", bufs=1) as cpool, \
             tc.tile_pool(name="rhsp", bufs=2) as rpool, \
             tc.tile_pool(name="scrp", bufs=4) as spool, \
             tc.tile_pool(name="accp", bufs=8, space="PSUM") as ppool:
            m_sb = {}
            for (K, idx), h in m_handles.items():
                mt = cpool.tile(list(h.shape), BF16, name=f"msb{K}_{idx}",
                                tag=f"m{K}_{idx}")
                nc.sync.dma_start(out=mt[:], in_=h[:])
                m_sb[(K, idx)] = mt
            fold_sb = cpool.tile([128, 8], F32, name="fold_sb", tag="fold")
            nc.sync.dma_start(out=fold_sb[:], in_=fold[:])
            stage = cpool.tile([128, TOT], F32, name="stage", tag="stage")

            for (K, T, scale) in convs:
                S, Q, U, chunks = _conv_cfg(K, T)
                qmaxT = (Q - 1) * T
                nb = S // BLOCK_I
                n = 2 * S
                nchunks = len(chunks)
                for g0 in range(0, nb, GROUP):
                    gblocks = list(range(g0, min(g0 + GROUP, nb)))
                    tiles = {}
                    psums = {}
                    for blk in gblocks:
                        psums[blk] = ppool.tile([128, n], F32,
                                                name=f"ps{K}_{blk}", tag="acc")
                    for q in range(Q):
                        u0 = qmaxT - q * T
                        for ci_, (lo, hi) in enumerate(chunks):
                            Kc = hi - lo
                            for blk in gblocks:
                                r0 = blk * BLOCK_I + q * T
                                key = (r0, ci_)
                                rt = tiles.get(key)
                                if rt is None:
                                    rt = rpool.tile(
                                        [T * Kc, n], BF16,
                                        name=f"r{K}_{r0}_{ci_}",
                                        tag=f"rhs{K}_{ci_}", bufs=RHS_BUFS[K])
                                    src = bass.AP(
                                        x, (r0 * IMG + lo) * 2,
                                        [[IMG * 2, T], [2, Kc], [1, n]])
                                    nc.gpsimd.dma_start(out=rt[:], in_=src)
                                    tiles[key] = rt
                                lhsT = m_sb[(K, ci_)].rearrange(
                                    "k (u o) -> k u o", o=16)[:, u0:u0 + 8, :]
                                nc.tensor.matmul(
                                    psums[blk][:], lhsT, rt[:],
                                    start=(q == 0 and ci_ == 0),
                                    stop=(q == Q - 1 and ci_ == nchunks - 1))
                    for blk in gblocks:
                        for b in range(2):
                            scr = spool.tile([128, S], F32,
                                             name=f"sq{K}_{blk}_{b}", tag="scr")
                            col = col_base[(K, b)] + blk
                            nc.scalar.activation(
                                out=scr[:],
                                in_=psums[blk][:, b::2],
                                func=mybir.ActivationFunctionType.Square,
                                accum_out=stage[:, col:col + 1])

            fold_ps = ppool.tile([8, TOT], F32, name="fold_ps", tag="acc")
            nc.tensor.matmul(fold_ps[:], fold_sb[:], stage[:],
                             start=True, stop=True)
            res = spool.tile([8, 8], F32, name="res", tag="res", bufs=1)
            for ci, (K, T, scale) in enumerate(CONVS):
                if (K, T, scale) not in convs:
                    continue
                nb = (IMG - K + 1) // BLOCK_I
                for b in range(2):
                    c0 = col_base[(K, b)]
                    oc = b * 4 + ci
                    nc.vector.reduce_sum(out=res[:8, oc:oc + 1],
                                         in_=fold_ps[:8, c0:c0 + nb],
                                         axis=mybir.AxisListType.X)
            dst = bass.AP(out, 0, [[4, 8], [32, 2], [1, 4]])
            nc.sync.dma_start(out=dst, in_=res[:8, :])
    return nc


_NC_CACHE = {}


def _get_nc(convs_key):
    if convs_key not in _NC_CACHE:
        nc = _build_nc(list(convs_key))
        nc.compile()
        _NC_CACHE[convs_key] = nc
    return _NC_CACHE[convs_key]


def kernel(x, w0, w1, w2, w3, _convs=None, _trace=False, _tmpdir=None):
    convs = CONVS if _convs is None else _convs
    ws = {25: w0, 49: w1, 97: w2, 193: w3}

    x = np.asarray(x, dtype=np.float32).reshape(16, IMG, IMG)
    xb = x.astype(ml_dtypes.bfloat16)

    shared = {}
    for (K, T, scale) in convs:
        w = np.asarray(ws[K], dtype=np.float32).reshape(16, K, K)
        for idx, M in enumerate(_build_M(w, K, T, scale)):
            shared[f"m{K}_{idx}"] = M.astype(ml_dtypes.bfloat16)
    shared["fold"] = _build_fold()

    in_maps = []
    for c in range(NCORES):
        m = dict(shared)
        m["x"] = np.ascontiguousarray(xb[2 * c:2 * c + 2].transpose(1, 2, 0))
        in_maps.append(m)

    nc = _get_nc(tuple(convs))
    kw = {}
    if _trace:
        kw.update(trace=True, tmpdir=_tmpdir)
    r = run_bass_kernel_spmd(nc, in_maps, list(range(NCORES)), **kw)
    out = np.concatenate([np.asarray(r.results[c]["out"], dtype=np.float32)
                          for c in range(NCORES)], axis=0)
    if _trace:
        kernel.last_exec_time_ns = r.exec_time_ns
        kernel.last_results = r
    return out



# revision 6
# speedup vs baseline: 129.6810x; 129.6810x over previous
"""Trainium2 Bass kernel for nn_Net_34763465294339.

Four single-channel VALID convs (K=25/49/97/193, 16 output channels each) on
x[16,1,256,256], each squared + spatially averaged / scale -> stack -> fold
16 channels into 8 by adding halves. Output [16,8,4] f32.

Sharding: data-parallel over batch, 2 images per core, weights replicated.

Resident-window conv (v2): x rows stay in DRAM in dense layout; per
output-row block a [rows, planes, cols] window tile is DMA'd with large
contiguous per-partition runs (no im2col gather). Kernel-column shifts are
expressed as overlapping column offsets in the matmul rhs AP; kernel-row
shifts live in zero-padded stationary weights (contraction over window rows).

  K=25/49/97 run in fp8e4 with perf_mode=DoubleRow: contraction packs
  (g-replica, row) on partitions x 2 interleave planes, giving 2G kernel
  columns (dj) per matmul. Window planes are pre-shifted by one column so the
  dj pair comes from the plane dim. Per-block/dj0 weights are AP slices of
  one padded matrix per conv ([(g,r), dj0, i, (u,o)] with u = 8*t + s for
  multi-block windows). The 1/(S^2*scale) factor is applied as the
  activation pre-scale (fp8 weights cannot be pre-scaled: underflow).

  K=193 runs in bf16 (fp8 error too large at P=64^2): two window tiles per
  block (rows 128 + 72), one matmul per (dj, half), pre-scaled weights.

Post: per block, ACT Square (scaled) with accum_out into a stage column; a
tiny fp32 fold-matmul adds the (s,o)->o%8 partitions; per-(conv,image)
column reduce; one DMA out.
"""
import numpy as np
import ml_dtypes

import concourse.bass as bass
import concourse.bacc as bacc
import concourse.mybir as mybir
from concourse.tile import TileContext
from concourse.bass_utils import run_bass_kernel_spmd

BF16 = mybir.dt.bfloat16
FP8 = mybir.dt.float8e4
F32 = mybir.dt.float32
NP_FP8 = ml_dtypes.float8_e4m3
NP_BF16 = ml_dtypes.bfloat16

IMG = 256
X8ROWS = 292  # padded rows for window reads past image end
NCORES = 8
BLOCK_I = 8

# fp8 convs: K -> (G, Rw, CH, ndj0, NBW, scale)
#   dj = CH*g + 2*dj0 + i ; window rows Rw serve NBW blocks (u = 8*t + s)
FP8_CONVS = {
    25: dict(G=2, Rw=64, CH=14, ndj0=7, NBW=4, scale=1.0),
    49: dict(G=2, Rw=64, CH=26, ndj0=13, NBW=2, scale=2.0),
    97: dict(G=1, Rw=104, CH=0, ndj0=49, NBW=1, scale=4.0),
}
K193_SCALE = 8.0
CONVS = [25, 49, 97, 193]


def _S(K):
    return IMG - K + 1


def _F(K):
    """fp8 window plane free size: max rhs read 4*(ndj0-1) + 2S, %16."""
    c = FP8_CONVS[K]
    f = 4 * (c['ndj0'] - 1) + 2 * _S(K)
    return (f + 15) // 16 * 16


def build_fp8_w(w, K):
    """w: [16,K,K] f32 raw. Returns [G*Rw, ndj0*2*U*16] fp8 where
    M[(g,r), dj0, i, (u,o)] = w[o, r-u, CH*g+2*dj0+i], U = 8*NBW."""
    c = FP8_CONVS[K]
    G, Rw, CH, ndj0, NBW = c['G'], c['Rw'], c['CH'], c['ndj0'], c['NBW']
    U = 8 * NBW
    M = np.zeros((G, Rw, ndj0, 2, U, 16), np.float32)
    r = np.arange(Rw)
    for g in range(G):
        for dj0 in range(ndj0):
            for i in range(2):
                dj = CH * g + 2 * dj0 + i
                if dj >= K:
                    continue
                for u in range(U):
                    valid = (r - u >= 0) & (r - u < K)
                    M[g, r[valid], dj0, i, u, :] = w[:, (r[valid] - u), dj].T
    return M.reshape(G * Rw, ndj0 * 2 * U * 16).astype(NP_FP8)


def build_w193(w):
    """w: [16,193,193] f32 pre-scaled. Returns (A [128, 193*128],
    B [72, 193*128]) bf16: A[p, dj, (s,o)] = w[o, p-s, dj],
    B[p, dj, (s,o)] = w[o, p+128-s, dj]."""
    A = np.zeros((128, 193, 8, 16), np.float32)
    B = np.zeros((72, 193, 8, 16), np.float32)
    for s in range(8):
        p = np.arange(128)
        d = p - s
        v = (d >= 0) & (d < 193)
        A[p[v], :, s, :] = w[:, d[v], :].transpose(1, 2, 0)
        p = np.arange(72)
        d = p + 128 - s
        v = (d >= 0) & (d < 193)
        B[p[v], :, s, :] = w[:, d[v], :].transpose(1, 2, 0)
    return (A.reshape(128, 193 * 128).astype(NP_BF16),
            B.reshape(72, 193 * 128).astype(NP_BF16))


def _build_fold():
    F = np.zeros((128, 8), dtype=np.float32)
    for p in range(128):
        F[p, (p % 16) % 8] = 1.0
    return F


def _col_layout():
    col_base = {}
    c = 0
    for K in CONVS:
        nb = _S(K) // BLOCK_I
        for b in range(2):
            col_base[(K, b)] = c
            c += nb
    return col_base, c


def build_in_maps(x, w0, w1, w2, w3):
    """Full inputs -> per-core input dicts for the compiled nc."""
    x = np.asarray(x, dtype=np.float32).reshape(16, IMG, IMG)
    ws = {25: w0, 49: w1, 97: w2, 193: w3}

    shared = {}
    for K in (25, 49, 97):
        w = np.asarray(ws[K], dtype=np.float32).reshape(16, K, K)
        shared[f"w{K}"] = build_fp8_w(w, K)
    w = np.asarray(ws[193], dtype=np.float32).reshape(16, 193, 193)
    w = w * np.sqrt(1.0 / (float(_S(193)) ** 2 * K193_SCALE), dtype=np.float32)
    shared["wA"], shared["wB"] = build_w193(w)
    shared["fold"] = _build_fold()

    in_maps = []
    for c in range(NCORES):
        m = dict(shared)
        # [row, (col, b)] interleaved pair of images
        pair = np.ascontiguousarray(
            x[2 * c:2 * c + 2].transpose(1, 2, 0)).reshape(IMG, 2 * IMG)
        x8 = np.zeros((X8ROWS, 2 * IMG), np.float32)
        x8[:IMG] = pair
        m["x8"] = x8.astype(NP_FP8)
        m["x16"] = pair.astype(NP_BF16)
        in_maps.append(m)
    return in_maps


def _build_nc(repeat=1):
    nc = bacc.Bacc("TRN2", target_bir_lowering=False)
    x8 = nc.dram_tensor("x8", [X8ROWS, 2 * IMG], FP8, kind="ExternalInput")
    x16 = nc.dram_tensor("x16", [IMG, 2 * IMG], BF16, kind="ExternalInput")
    w_h = {}
    for K in (25, 49, 97):
        c = FP8_CONVS[K]
        w_h[K] = nc.dram_tensor(
            f"w{K}", [c['G'] * c['Rw'], c['ndj0'] * 2 * 8 * c['NBW'] * 16],
            FP8, kind="ExternalInput")
    wA_h = nc.dram_tensor("wA", [128, 193 * 128], BF16, kind="ExternalInput")
    wB_h = nc.dram_tensor("wB", [72, 193 * 128], BF16, kind="ExternalInput")
    fold_h = nc.dram_tensor("fold", [128, 8], F32, kind="ExternalInput")
    out = nc.dram_tensor("out", [2, 8, 4], F32, kind="ExternalOutput")

    col_base, TOT = _col_layout()
    SQ = mybir.ActivationFunctionType.Square
    DR = mybir.MatmulPerfMode.DoubleRow

    with TileContext(nc) as tc:
        with tc.tile_pool(name="consts", bufs=1) as cpool, \
             tc.tile_pool(name="winp", bufs=2) as rpool, \
             tc.tile_pool(name="scrp", bufs=4) as spool, \
             tc.tile_pool(name="accp", bufs=8, space="PSUM") as ppool:
            w_sb = {}
            for K in (25, 49, 97):
                t = cpool.tile(list(w_h[K].shape), FP8, name=f"w{K}sb",
                               tag=f"w{K}")
                nc.sync.dma_start(out=t[:], in_=w_h[K][:])
                w_sb[K] = t
            wA = cpool.tile([128, 193 * 128], BF16, name="wAsb", tag="wA")
            nc.sync.dma_start(out=wA[:], in_=wA_h[:])
            wB = cpool.tile([72, 193 * 128], BF16, name="wBsb", tag="wB")
            nc.sync.dma_start(out=wB[:], in_=wB_h[:])
            fold_sb = cpool.tile([128, 8], F32, name="fold_sb", tag="fold")
            nc.sync.dma_start(out=fold_sb[:], in_=fold_h[:])
            stage = cpool.tile([128, TOT], F32, name="stage", tag="stage")

            rep = tc.For_i(0, repeat) if repeat != 1 else None
            if rep is not None:
                rep.__enter__()

            # fp8 DoubleRow convs
            for K in (25, 49, 97):
                c = FP8_CONVS[K]
                G, Rw, CH, ndj0, NBW, scale = (
                    c['G'], c['Rw'], c['CH'], c['ndj0'], c['NBW'], c['scale'])
                S = _S(K)
                F = _F(K)
                U = 8 * NBW
                nb = S // BLOCK_I
                act_scale = float(np.sqrt(1.0 / (float(S) ** 2 * scale)))
                nwin = (nb + NBW - 1) // NBW
                for wi in range(nwin):
                    i0 = wi * NBW * BLOCK_I
                    nt = min(NBW, nb - wi * NBW)
                    win = rpool.tile([G * Rw, 2 * F], FP8,
                                     name=f"win{K}_{wi}", tag=f"win{K}",
                                     bufs=4)
                    for g in range(G):
                        src = bass.AP(
                            x8, i0 * 2 * IMG + 2 * CH * g,
                            [[2 * IMG, Rw], [2, 2], [1, F]])
                        nc.sync.dma_start(out=win[g * Rw:(g + 1) * Rw, :],
                                          in_=src)
                    win3 = win.rearrange("p (i f) -> p i f", i=2)
                    wm = w_sb[K].rearrange("p (d i m) -> p d i m",
                                           d=ndj0, i=2)
                    psums = [ppool.tile([128, 2 * S], F32,
                                        name=f"ps{K}_{wi}_{t}", tag="acc")
                             for t in range(nt)]
                    for dj0 in range(ndj0):
                        rhs = win3[:, :, 4 * dj0:4 * dj0 + 2 * S]
                        for t in range(nt):
                            lhsT = wm[:, dj0, :, 8 * t * 16:8 * t * 16 + 128]
                            nc.tensor.matmul(
                                psums[t][:], lhsT, rhs,
                                start=(dj0 == 0), stop=(dj0 == ndj0 - 1),
                                perf_mode=DR)
                    for t in range(nt):
                        blk = wi * NBW + t
                        for b in range(2):
                            scr = spool.tile([128, S], F32,
                                             name=f"sq{K}_{blk}_{b}",
                                             tag="scr")
                            col = col_base[(K, b)] + blk
                            nc.scalar.activation(
                                out=scr[:], in_=psums[t][:, b::2], func=SQ,
                                scale=act_scale,
                                accum_out=stage[:, col:col + 1])

            # bf16 conv K=193
            S = _S(193)
            nb = S // BLOCK_I
            wA3 = wA.rearrange("p (d m) -> p d m", m=128)
            wB3 = wB.rearrange("p (d m) -> p d m", m=128)
            for blk in range(nb):
                i0 = blk * BLOCK_I
                winA = rpool.tile([128, 2 * IMG], BF16,
                                  name=f"winA_{blk}", tag="winA", bufs=3)
                src = bass.AP(x16, i0 * 2 * IMG, [[2 * IMG, 128], [1, 2 * IMG]])
                nc.sync.dma_start(out=winA[:], in_=src)
                winB = rpool.tile([72, 2 * IMG], BF16,
                                  name=f"winB_{blk}", tag="winB", bufs=3)
                src = bass.AP(x16, (i0 + 128) * 2 * IMG,
                              [[2 * IMG, 72], [1, 2 * IMG]])
                nc.sync.dma_start(out=winB[:], in_=src)
                ps = ppool.tile([128, 2 * S], F32, name=f"ps193_{blk}",
                                tag="acc")
                for dj in range(193):
                    nc.tensor.matmul(ps[:], wA3[:, dj, :],
                                     winA[:, 2 * dj:2 * dj + 2 * S],
                                     start=(dj == 0), stop=False)
                    nc.tensor.matmul(ps[:], wB3[:, dj, :],
                                     winB[:, 2 * dj:2 * dj + 2 * S],
                                     start=False, stop=(dj == 192))
                for b in range(2):
                    scr = spool.tile([128, S], F32, name=f"sq193_{blk}_{b}",
                                     tag="scr")
                    col = col_base[(193, b)] + blk
                    nc.scalar.activation(
                        out=scr[:], in_=ps[:, b::2], func=SQ,
                        accum_out=stage[:, col:col + 1])

            # fold (s,o) partitions -> o%8, then per-(conv,image) reduce
            fold_ps = ppool.tile([8, TOT], F32, name="fold_ps", tag="acc")
            nc.tensor.matmul(fold_ps[:], fold_sb[:], stage[:],
                             start=True, stop=True)
            res = spool.tile([8, 8], F32, name="res", tag="res", bufs=1)
            for ci, K in enumerate(CONVS):
                nb = _S(K) // BLOCK_I
                for b in range(2):
                    c0 = col_base[(K, b)]
                    oc = b * 4 + ci
                    nc.vector.reduce_sum(out=res[:8, oc:oc + 1],
                                         in_=fold_ps[:8, c0:c0 + nb],
                                         axis=mybir.AxisListType.X)
            dst = bass.AP(out, 0, [[4, 8], [32, 2], [1, 4]])
            nc.sync.dma_start(out=dst, in_=res[:8, :])
            if rep is not None:
                rep.__exit__(None, None, None)
    return nc


_NC_CACHE = {}


def _get_nc(repeat=1):
    if repeat not in _NC_CACHE:
        nc = _build_nc(repeat=repeat)
        nc.compile()
        _NC_CACHE[repeat] = nc
    return _NC_CACHE[repeat]


def kernel(x, w0, w1, w2, w3):
    in_maps = build_in_maps(x, w0, w1, w2, w3)
    nc = _get_nc()
    r = run_bass_kernel_spmd(nc, in_maps, list(range(NCORES)))
    return np.concatenate([np.asarray(r.results[c]["out"], dtype=np.float32)
                           for c in range(NCORES)], axis=0)


# revision 14
# speedup vs baseline: 181.9259x; 1.4029x over previous
"""Trainium2 Bass kernel for nn_Net_34763465294339.

Four single-channel VALID convs (K=25/49/97/193, 16 output channels each) on
x[16,1,256,256], each squared + spatially averaged / scale -> stack -> fold
16 channels into 8 by adding halves. Output [16,8,4] f32.

Sharding: data-parallel over batch, 2 images per core, weights replicated.

Resident-window conv (v2): x rows stay in DRAM in dense layout; per
output-row block a [rows, planes, cols] window tile is DMA'd with large
contiguous per-partition runs (no im2col gather). Kernel-column shifts are
expressed as overlapping column offsets in the matmul rhs AP; kernel-row
shifts live in zero-padded stationary weights (contraction over window rows).

  K=25/49/97 run in fp8e4 with perf_mode=DoubleRow: contraction packs
  (g-replica, row) on partitions x 2 interleave planes, giving 2G kernel
  columns (dj) per matmul. Window planes are pre-shifted by one column so the
  dj pair comes from the plane dim. Per-block/dj0 weights are AP slices of
  one padded matrix per conv ([(g,r), dj0, i, (u,o)] with u = 8*t + s for
  multi-block windows). The 1/(S^2*scale) factor is applied as the
  activation pre-scale (fp8 weights cannot be pre-scaled: underflow).

  K=193 runs in bf16 (fp8 error too large at P=64^2): two window tiles per
  block (rows 128 + 72), one matmul per (dj, half), pre-scaled weights.
  To halve its matmul count (N=128 matmuls are issue/LDW-floor-bound),
  conv193 is resharded: cores pair up per 4-image group, each core runs the
  SAME program blocks {0..3} on x16g whose content is row-shifted per core
  (even core of a pair: rows 0..223 of the group's 4 images; odd: rows
  32..255), so N = 4 img * 64 = 256 and each core emits per-image partial
  energies (out193) that the host sums across the pair.

Post: per block, ACT Square (scaled) with accum_out into a stage column; a
tiny fp32 fold-matmul adds the (s,o)->o%8 partitions; per-(conv,image)
column reduce; one DMA out.
"""
import numpy as np
import ml_dtypes

import concourse.bass as bass
import concourse.bacc as bacc
import concourse.mybir as mybir
from concourse.tile import TileContext
from concourse.bass_utils import run_bass_kernel_spmd

BF16 = mybir.dt.bfloat16
FP8 = mybir.dt.float8e4
F32 = mybir.dt.float32
NP_FP8 = ml_dtypes.float8_e4m3
NP_BF16 = ml_dtypes.bfloat16

IMG = 256
X8ROWS = 292  # padded rows for window reads past image end
NCORES = 8
BLOCK_I = 8

# fp8 convs: K -> (G, Rw, CH, ndj0, NBW, scale)
#   dj = CH*g + 2*dj0 + i ; window rows Rw serve NBW blocks (u = 8*t + s)
FP8_CONVS = {
    25: dict(G=2, Rw=64, CH=14, ndj0=7, NBW=4, scale=1.0),
    49: dict(G=2, Rw=64, CH=26, ndj0=13, NBW=2, scale=2.0),
    97: dict(G=1, Rw=104, CH=0, ndj0=49, NBW=1, scale=4.0),
}
K193_SCALE = 8.0
CONVS = [25, 49, 97, 193]


def _S(K):
    return IMG - K + 1


def _F(K):
    """fp8 window plane free size: max rhs read 4*(ndj0-1) + 2S, %16."""
    c = FP8_CONVS[K]
    f = 4 * (c['ndj0'] - 1) + 2 * _S(K)
    return (f + 15) // 16 * 16


def build_fp8_w(w, K):
    """w: [16,K,K] f32 raw. Returns [G*Rw, ndj0*2*U*16] fp8 where
    M[(g,r), dj0, i, (u,o)] = w[o, r-u, CH*g+2*dj0+i], U = 8*NBW."""
    c = FP8_CONVS[K]
    G, Rw, CH, ndj0, NBW = c['G'], c['Rw'], c['CH'], c['ndj0'], c['NBW']
    U = 8 * NBW
    M = np.zeros((G, Rw, ndj0, 2, U, 16), np.float32)
    r = np.arange(Rw)
    for g in range(G):
        for dj0 in range(ndj0):
            for i in range(2):
                dj = CH * g + 2 * dj0 + i
                if dj >= K:
                    continue
                for u in range(U):
                    valid = (r - u >= 0) & (r - u < K)
                    M[g, r[valid], dj0, i, u, :] = w[:, (r[valid] - u), dj].T
    return M.reshape(G * Rw, ndj0 * 2 * U * 16).astype(NP_FP8)


def build_w193(w):
    """w: [16,193,193] f32 pre-scaled. Returns (A [128, 193*128],
    B [72, 193*128]) bf16: A[p, dj, (s,o)] = w[o, p-s, dj],
    B[p, dj, (s,o)] = w[o, p+128-s, dj]."""
    A = np.zeros((128, 193, 8, 16), np.float32)
    B = np.zeros((72, 193, 8, 16), np.float32)
    for s in range(8):
        p = np.arange(128)
        d = p - s
        v = (d >= 0) & (d < 193)
        A[p[v], :, s, :] = w[:, d[v], :].transpose(1, 2, 0)
        p = np.arange(72)
        d = p + 128 - s
        v = (d >= 0) & (d < 193)
        B[p[v], :, s, :] = w[:, d[v], :].transpose(1, 2, 0)
    return (A.reshape(128, 193 * 128).astype(NP_BF16),
            B.reshape(72, 193 * 128).astype(NP_BF16))


def _build_fold():
    F = np.zeros((128, 8), dtype=np.float32)
    for p in range(128):
        F[p, (p % 16) % 8] = 1.0
    return F


def _col_layout():
    """fp8 convs: (K, b) -> base col, width nb. conv193: (193,) -> base col,
    then col = base + img*4 + blk (4 imgs x 4 blocks)."""
    col_base = {}
    c = 0
    for K in (25, 49, 97):
        nb = _S(K) // BLOCK_I
        for b in range(2):
            col_base[(K, b)] = c
            c += nb
    col_base[(193,)] = c
    c += 16
    return col_base, c


def build_in_maps(x, w0, w1, w2, w3):
    """Full inputs -> per-core input dicts for the compiled nc."""
    x = np.asarray(x, dtype=np.float32).reshape(16, IMG, IMG)
    ws = {25: w0, 49: w1, 97: w2, 193: w3}

    shared = {}
    for K in (25, 49, 97):
        w = np.asarray(ws[K], dtype=np.float32).reshape(16, K, K)
        shared[f"w{K}"] = build_fp8_w(w, K)
    w = np.asarray(ws[193], dtype=np.float32).reshape(16, 193, 193)
    w = w * np.sqrt(1.0 / (float(_S(193)) ** 2 * K193_SCALE), dtype=np.float32)
    shared["wA"], shared["wB"] = build_w193(w)
    shared["fold"] = _build_fold()

    in_maps = []
    for c in range(NCORES):
        m = dict(shared)
        # [row, (col, b)] interleaved pair of images
        pair = np.ascontiguousarray(
            x[2 * c:2 * c + 2].transpose(1, 2, 0)).reshape(IMG, 2 * IMG)
        x8 = np.zeros((X8ROWS, 2 * IMG), np.float32)
        x8[:IMG] = pair
        m["x8"] = x8.astype(NP_FP8)
        # conv193 group input: 4 images of group c//2, row-shifted by
        # 32*(c%2) so program blocks {0..3} compute real blocks {0..3}/{4..7}
        g = c // 2
        quad = np.ascontiguousarray(
            x[4 * g:4 * g + 4].transpose(1, 2, 0)).reshape(IMG, 4 * IMG)
        r0 = 32 * (c % 2)
        m["x16g"] = np.ascontiguousarray(
            quad[r0:r0 + 224]).astype(NP_BF16)
        in_maps.append(m)
    return in_maps


def _build_nc(repeat=1):
    nc = bacc.Bacc("TRN2", target_bir_lowering=False)
    x8 = nc.dram_tensor("x8", [X8ROWS, 2 * IMG], FP8, kind="ExternalInput")
    x16g = nc.dram_tensor("x16g", [224, 4 * IMG], BF16, kind="ExternalInput")
    w_h = {}
    for K in (25, 49, 97):
        c = FP8_CONVS[K]
        w_h[K] = nc.dram_tensor(
            f"w{K}", [c['G'] * c['Rw'], c['ndj0'] * 2 * 8 * c['NBW'] * 16],
            FP8, kind="ExternalInput")
    wA_h = nc.dram_tensor("wA", [128, 193 * 128], BF16, kind="ExternalInput")
    wB_h = nc.dram_tensor("wB", [72, 193 * 128], BF16, kind="ExternalInput")
    fold_h = nc.dram_tensor("fold", [128, 8], F32, kind="ExternalInput")
    out = nc.dram_tensor("out", [2, 8, 4], F32, kind="ExternalOutput")
    out193 = nc.dram_tensor("out193", [4, 8], F32, kind="ExternalOutput")

    col_base, TOT = _col_layout()
    SQ = mybir.ActivationFunctionType.Square
    DR = mybir.MatmulPerfMode.DoubleRow

    with TileContext(nc) as tc:
        with tc.tile_pool(name="consts", bufs=1) as cpool, \
             tc.tile_pool(name="winp", bufs=2) as rpool, \
             tc.tile_pool(name="scrp", bufs=4) as spool, \
             tc.tile_pool(name="accp", bufs=8, space="PSUM") as ppool:
            w_sb = {}
            for K in (25, 49, 97):
                t = cpool.tile(list(w_h[K].shape), FP8, name=f"w{K}sb",
                               tag=f"w{K}")
                nc.sync.dma_start(out=t[:], in_=w_h[K][:])
                w_sb[K] = t
            wA = cpool.tile([128, 193 * 128], BF16, name="wAsb", tag="wA")
            nc.sync.dma_start(out=wA[:], in_=wA_h[:])
            wB = cpool.tile([72, 193 * 128], BF16, name="wBsb", tag="wB")
            nc.sync.dma_start(out=wB[:], in_=wB_h[:])
            fold_sb = cpool.tile([128, 8], F32, name="fold_sb", tag="fold")
            nc.sync.dma_start(out=fold_sb[:], in_=fold_h[:])
            stage = cpool.tile([128, TOT], F32, name="stage", tag="stage")

            rep = tc.For_i(0, repeat) if repeat != 1 else None
            if rep is not None:
                rep.__enter__()

            # fp8 DoubleRow convs
            for K in (25, 49, 97):
                c = FP8_CONVS[K]
                G, Rw, CH, ndj0, NBW, scale = (
                    c['G'], c['Rw'], c['CH'], c['ndj0'], c['NBW'], c['scale'])
                S = _S(K)
                F = _F(K)
                U = 8 * NBW
                nb = S // BLOCK_I
                act_scale = float(np.sqrt(1.0 / (float(S) ** 2 * scale)))
                nwin = (nb + NBW - 1) // NBW
                for wi in range(nwin):
                    i0 = wi * NBW * BLOCK_I
                    nt = min(NBW, nb - wi * NBW)
                    win = rpool.tile([G * Rw, 2 * F], FP8,
                                     name=f"win{K}_{wi}", tag=f"win{K}",
                                     bufs=4)
                    for g in range(G):
                        src = bass.AP(
                            x8, i0 * 2 * IMG + 2 * CH * g,
                            [[2 * IMG, Rw], [2, 2], [1, F]])
                        nc.sync.dma_start(out=win[g * Rw:(g + 1) * Rw, :],
                                          in_=src)
                    win3 = win.rearrange("p (i f) -> p i f", i=2)
                    wm = w_sb[K].rearrange("p (d i m) -> p d i m",
                                           d=ndj0, i=2)
                    psums = [ppool.tile([128, 2 * S], F32,
                                        name=f"ps{K}_{wi}_{t}", tag="acc")
                             for t in range(nt)]
                    for dj0 in range(ndj0):
                        rhs = win3[:, :, 4 * dj0:4 * dj0 + 2 * S]
                        for t in range(nt):
                            lhsT = wm[:, dj0, :, 8 * t * 16:8 * t * 16 + 128]
                            nc.tensor.matmul(
                                psums[t][:], lhsT, rhs,
                                start=(dj0 == 0), stop=(dj0 == ndj0 - 1),
                                perf_mode=DR)
                    for t in range(nt):
                        blk = wi * NBW + t
                        for b in range(2):
                            scr = spool.tile([128, S], F32,
                                             name=f"sq{K}_{blk}_{b}",
                                             tag="scr")
                            col = col_base[(K, b)] + blk
                            nc.scalar.activation(
                                out=scr[:], in_=psums[t][:, b::2], func=SQ,
                                scale=act_scale,
                                accum_out=stage[:, col:col + 1])

            # bf16 conv K=193: 4 program blocks x 4 group images (N=256)
            S = _S(193)
            wA3 = wA.rearrange("p (d m) -> p d m", m=128)
            wB3 = wB.rearrange("p (d m) -> p d m", m=128)
            for blk in range(4):
                i0 = blk * BLOCK_I
                winA = rpool.tile([128, 4 * IMG], BF16,
                                  name=f"winA_{blk}", tag="winA", bufs=3)
                src = bass.AP(x16g, i0 * 4 * IMG,
                              [[4 * IMG, 128], [1, 4 * IMG]])
                nc.sync.dma_start(out=winA[:], in_=src)
                winB = rpool.tile([72, 4 * IMG], BF16,
                                  name=f"winB_{blk}", tag="winB", bufs=3)
                src = bass.AP(x16g, (i0 + 128) * 4 * IMG,
                              [[4 * IMG, 72], [1, 4 * IMG]])
                nc.sync.dma_start(out=winB[:], in_=src)
                ps = ppool.tile([128, 4 * S], F32, name=f"ps193_{blk}",
                                tag="acc")
                for dj in range(193):
                    nc.tensor.matmul(ps[:], wA3[:, dj, :],
                                     winA[:, 4 * dj:4 * dj + 4 * S],
                                     start=(dj == 0), stop=False)
                    nc.tensor.matmul(ps[:], wB3[:, dj, :],
                                     winB[:, 4 * dj:4 * dj + 4 * S],
                                     start=False, stop=(dj == 192))
                for b in range(4):
                    scr = spool.tile([128, S], F32, name=f"sq193_{blk}_{b}",
                                     tag="scr")
                    col = col_base[(193,)] + b * 4 + blk
                    nc.scalar.activation(
                        out=scr[:], in_=ps[:, b::4], func=SQ,
                        accum_out=stage[:, col:col + 1])

            # fold (s,o) partitions -> o%8, then per-(conv,image) reduce
            fold_ps = ppool.tile([8, TOT], F32, name="fold_ps", tag="acc")
            nc.tensor.matmul(fold_ps[:], fold_sb[:], stage[:],
                             start=True, stop=True)
            res = spool.tile([8, 8], F32, name="res", tag="res", bufs=1)
            for ci, K in enumerate((25, 49, 97)):
                nb = _S(K) // BLOCK_I
                for b in range(2):
                    c0 = col_base[(K, b)]
                    oc = b * 4 + ci
                    nc.vector.reduce_sum(out=res[:8, oc:oc + 1],
                                         in_=fold_ps[:8, c0:c0 + nb],
                                         axis=mybir.AxisListType.X)
            # conv193 per-group-image partials (summed across core pair on
            # host); also park finite filler in res cols 3/7 (host ignores)
            res193 = spool.tile([8, 4], F32, name="res193", tag="res193",
                                bufs=1)
            c0 = col_base[(193,)]
            for i in range(4):
                nc.vector.reduce_sum(out=res193[:8, i:i + 1],
                                     in_=fold_ps[:8, c0 + 4 * i:c0 + 4 * i + 4],
                                     axis=mybir.AxisListType.X)
            for oc in (3, 7):
                nc.vector.reduce_sum(out=res[:8, oc:oc + 1],
                                     in_=fold_ps[:8, c0:c0 + 4],
                                     axis=mybir.AxisListType.X)
            dst = bass.AP(out, 0, [[4, 8], [32, 2], [1, 4]])
            nc.sync.dma_start(out=dst, in_=res[:8, :])
            dst193 = bass.AP(out193, 0, [[1, 8], [8, 4]])
            nc.sync.dma_start(out=dst193, in_=res193[:8, :])
            if rep is not None:
                rep.__exit__(None, None, None)
    return nc


_NC_CACHE = {}


def _get_nc(repeat=1):
    if repeat not in _NC_CACHE:
        nc = _build_nc(repeat=repeat)
        nc.compile()
        _NC_CACHE[repeat] = nc
    return _NC_CACHE[repeat]


def kernel(x, w0, w1, w2, w3):
    in_maps = build_in_maps(x, w0, w1, w2, w3)
    nc = _get_nc()
    r = run_bass_kernel_spmd(nc, in_maps, list(range(NCORES)))
    final = np.concatenate([np.asarray(r.results[c]["out"], dtype=np.float32)
                            for c in range(NCORES)], axis=0)
    for g in range(4):
        p = (np.asarray(r.results[2 * g]["out193"], dtype=np.float32)
             + np.asarray(r.results[2 * g + 1]["out193"], dtype=np.float32))
        final[4 * g:4 * g + 4, :, 3] = p
    return final
